# revision 1
# baseline (speedup 1.0000x reference)
"""Expert-parallel MoE BaseLayer kernel for 8 Trainium2 NeuronCores.

Strategy (per the expert-parallel sharding hint):
  - Host: route tokens by argmax affinity (float64 numpy - the top-2 gaps are
    >>fp32 noise so this reproduces the reference's fp32 argmax), compute the
    sigmoid gate alpha on host, sort tokens by expert, pad each expert group
    to a common capacity C (multiple of 128).
  - Device (one Bass program, SPMD over 8 cores; core e holds expert e):
      LayerNorm (token-major) -> bf16 -> DRAM bounce -> XBAR-transposed load
      (D-major) -> ff1 (h^T = w1^T @ xln^T, PSUM-accumulated) -> relu+b1 ->
      ff2 (ffn = h @ w2, PSUM-accumulated) -> out = x + alpha * (ffn + b2).
    Matmuls run in bf16 with fp32 PSUM accumulation.
  - Host: scatter per-expert outputs back to the original token order.
"""

import os

import numpy as np
import ml_dtypes

B, S, D, F, E = 8, 1024, 1024, 4096, 8
T = B * S
EPS = 1e-5
P = 128
CHUNK = 384  # tokens per pipeline chunk (<=512 for PSUM; 3 token-tiles)

_NC_CACHE = {}
LAST_EXEC_TIME_NS = None
LAST_RESULTS = None


def _chunk_sizes(C):
    sizes = [CHUNK] * (C // CHUNK)
    if C % CHUNK:
        sizes.append(C % CHUNK)
    assert sum(sizes) == C and all(s % P == 0 for s in sizes)
    return sizes


def _build_nc(C, apply_gb, apply_b1, apply_b2):
    import concourse.bass as bass
    import concourse.tile as tile
    from concourse import bacc, mybir
    from concourse.bass import ts
    from concourse.masks import make_identity

    f32 = mybir.dt.float32
    bf16 = mybir.dt.bfloat16

    KD = D // P    # 8 k-tiles over D
    MF = F // P    # 32 f-tiles over F
    ND = D // 512  # 2 n-tiles over D for ff2
    n_tok_tiles = C // P
    chunks = _chunk_sizes(C)

    nc = bacc.Bacc()
    x_in = nc.declare_dram_parameter("x", [C, D], f32, isOutput=False)
    w1_in = nc.declare_dram_parameter("w1", [D, F], bf16, isOutput=False)
    w2_in = nc.declare_dram_parameter("w2", [F, D], bf16, isOutput=False)
    alpha_in = nc.declare_dram_parameter("alpha_t", [P, n_tok_tiles], f32, isOutput=False)
    if apply_b1:
        b1_in = nc.declare_dram_parameter("b1_t", [P, MF], f32, isOutput=False)
    if apply_gb:
        g_in = nc.declare_dram_parameter("g_t", [P, KD], f32, isOutput=False)
        bb_in = nc.declare_dram_parameter("b_t", [P, KD], f32, isOutput=False)
    if apply_b2:
        b2_in = nc.declare_dram_parameter("b2", [1, D], f32, isOutput=False)
    out_ext = nc.declare_dram_parameter("out", [C, D], f32, isOutput=True)

    x_tiles = x_in[:].rearrange("(t p) d -> t p d", p=P)
    out_tiles = out_ext[:].rearrange("(t p) d -> t p d", p=P)
    w1_view = w1_in[:].rearrange("(k p) f -> k p f", p=P)
    w2_view = w2_in[:].rearrange("(k p) d -> k p d", p=P)

    with tile.TileContext(nc) as tc:
        from contextlib import ExitStack

        with ExitStack() as ctx:
            singles = ctx.enter_context(tc.tile_pool(name="singles", bufs=1))
            xa_pool = ctx.enter_context(tc.tile_pool(name="xa", bufs=2))
            xn_pool = ctx.enter_context(tc.tile_pool(name="xn", bufs=2))
            st_pool = ctx.enter_context(tc.tile_pool(name="stats", bufs=4))
            xlnt_pool = ctx.enter_context(tc.tile_pool(name="xlnt", bufs=1))
            ht_pool = ctx.enter_context(tc.tile_pool(name="ht", bufs=1))
            xd_pool = ctx.enter_context(tc.tile_pool(name="xd", bufs=2))
            out_pool = ctx.enter_context(tc.tile_pool(name="outp", bufs=2))
            psA = ctx.enter_context(tc.tile_pool(name="psA", bufs=2, space="PSUM"))
            psT = ctx.enter_context(tc.tile_pool(name="psT", bufs=3, space="PSUM"))
            xf_pool = ctx.enter_context(tc.tile_pool(name="xf32", bufs=2))
            psB = ctx.enter_context(tc.tile_pool(name="psB", bufs=3, space="PSUM"))
            dram = ctx.enter_context(tc.tile_pool(name="dram", bufs=1, space="DRAM"))

            # --- small resident constants (cheap DMAs first) ------------
            alpha_sb = singles.tile([P, n_tok_tiles], f32)
            nc.sync.dma_start(out=alpha_sb[:], in_=alpha_in[:])
            eps_sb = singles.tile([P, 1], f32)
            nc.vector.memset(eps_sb, EPS)
            if apply_b1:
                b1_sb = singles.tile([P, MF], f32)
                nc.sync.dma_start(out=b1_sb[:], in_=b1_in[:])
            if apply_gb:
                g_sb = singles.tile([P, KD], f32)
                nc.sync.dma_start(out=g_sb[:], in_=g_in[:])
                b_sb = singles.tile([P, KD], f32)
                nc.sync.dma_start(out=b_sb[:], in_=bb_in[:])
            if apply_b2:
                b2_sb = singles.tile([P, D], f32)
                nc.sync.dma_start(out=b2_sb[:], in_=b2_in[:].to_broadcast((P, D)))

            w1_sb = singles.tile([P, KD, F], bf16)
            w2_sb = singles.tile([P, MF, D], bf16)
            ident = singles.tile([P, P], f32)
            make_identity(nc, ident[:])

            # --- phase 0: LayerNorm + transpose for every chunk ---------
            # chunk 0's LN pipeline is emitted before the w1 bulk load so
            # its DMAs are not queued behind 8 MB of weights.
            xlnT = {}
            c0 = 0
            for ci, Cc in enumerate(chunks):
                pe_transpose = ci == 0
                if not pe_transpose:
                    xn_dram = dram.tile([Cc, D], bf16, tag=f"xnd{ci}")
                    xn_dview = xn_dram[:].rearrange("c (k p) -> c k p", p=P)
                xlnT_c = xlnt_pool.tile([P, KD, Cc], bf16, tag=f"xlnt{ci}")
                for tloc in range(Cc // P):
                    ti = c0 // P + tloc
                    x_sb = xa_pool.tile([P, D], f32)
                    nc.sync.dma_start(out=x_sb[:, :512], in_=x_tiles[ti][:, :512])
                    nc.sync.dma_start(out=x_sb[:, 512:], in_=x_tiles[ti][:, 512:])
                    stats = st_pool.tile([P, 2, 6], f32)
                    x_grp = x_sb[:].rearrange("p (s q) -> p s q", q=512)
                    for s in range(2):
                        nc.vector.bn_stats(out=stats[:, s, :], in_=x_grp[:, s, :])
                    mv = st_pool.tile([P, 2], f32)
                    nc.vector.bn_aggr(out=mv[:], in_=stats[:])
                    rstd = st_pool.tile([P, 1], f32)
                    nc.scalar.activation(
                        out=rstd[:],
                        in_=mv[:, 1:2],
                        func=mybir.ActivationFunctionType.Sqrt,
                        bias=eps_sb[:],
                        scale=1.0,
                    )
                    nc.vector.reciprocal(out=rstd[:], in_=rstd[:])
                    if pe_transpose:
                        # chunk 0: transpose on the (idle) PE instead of the
                        # DRAM bounce - keeps the ramp off the DMA queues.
                        xn32 = xf_pool.tile([P, D], f32)
                        nc.vector.tensor_scalar(
                            out=xn32[:],
                            in0=x_sb[:],
                            scalar1=mv[:, 0:1],
                            scalar2=rstd[:],
                            op0=mybir.AluOpType.subtract,
                            op1=mybir.AluOpType.mult,
                        )
                        for k in range(KD):
                            tps = psT.tile([P, P], f32, tag="psT")
                            nc.tensor.transpose(
                                tps[:], xn32[:, ts(k, P)], ident[:]
                            )
                            nc.vector.tensor_copy(
                                out=xlnT_c[:, k, tloc * P:(tloc + 1) * P],
                                in_=tps[:],
                            )
                    else:
                        xn_sb = xn_pool.tile([P, D], bf16)
                        nc.vector.tensor_scalar(
                            out=xn_sb[:],
                            in0=x_sb[:],
                            scalar1=mv[:, 0:1],
                            scalar2=rstd[:],
                            op0=mybir.AluOpType.subtract,
                            op1=mybir.AluOpType.mult,
                        )
                        nc.sync.dma_start(
                            out=xn_dram[tloc * P:(tloc + 1) * P, :], in_=xn_sb[:]
                        )
                if not pe_transpose:
                    # transposed load: [Cc, 128] -> [128, Cc] per D-tile
                    for k in range(KD):
                        nc.sync.dma_start(
                            out=xlnT_c[:, k, :], in_=xn_dview[:, k], transpose=True
                        )
                if apply_gb:
                    for k in range(KD):
                        nc.vector.tensor_scalar(
                            out=xlnT_c[:, k, :],
                            in0=xlnT_c[:, k, :],
                            scalar1=g_sb[:, k:k + 1],
                            scalar2=b_sb[:, k:k + 1],
                            op0=mybir.AluOpType.mult,
                            op1=mybir.AluOpType.add,
                        )
                xlnT[ci] = xlnT_c
                c0 += Cc
                if ci == 0:
                    # weight bulk loads after chunk 0's LN DMAs. w1 arrives in
                    # m-quarters (all k-rows of m 0..7 first, ...) so ff1's
                    # early m-sweeps start before the full 8 MB has landed.
                    FQ = F // 4
                    for q in range(4):
                        for k in range(KD):
                            nc.sync.dma_start(
                                out=w1_sb[:, k, q * FQ:(q + 1) * FQ],
                                in_=w1_view[k][:, q * FQ:(q + 1) * FQ],
                            )
                    for k in range(MF):
                        nc.sync.dma_start(out=w2_sb[:, k, :], in_=w2_view[k])

            # --- per chunk: ff1 -> relu -> ff2 -> combine ---------------
            c0 = 0
            for ci, Cc in enumerate(chunks):
                n_mt = Cc // P
                # ff1: h^T[f, t] for this chunk
                hT = ht_pool.tile([P, MF, CHUNK], bf16, tag="ht")
                for m in range(MF):
                    ps = psA.tile([P, 512], f32, tag="psA")
                    for k in range(KD):
                        nc.tensor.matmul(
                            ps[:, :Cc],
                            lhsT=w1_sb[:, k, ts(m, P)],
                            rhs=xlnT[ci][:, k, :],
                            start=(k == 0),
                            stop=(k == KD - 1),
                        )
                    nc.scalar.activation(
                        out=hT[:, m, :Cc],
                        in_=ps[:, :Cc],
                        func=mybir.ActivationFunctionType.Relu,
                        bias=(b1_sb[:, m:m + 1] if apply_b1 else 0.0),
                        scale=1.0,
                    )

                # ff2 + combine, per 128-token tile: out = x + alpha*(ffn+b2)
                for mt in range(n_mt):
                    gti = c0 // P + mt
                    xd = xd_pool.tile([P, D], f32)
                    nc.sync.dma_start(out=xd[:], in_=x_tiles[gti])
                    o_sb = out_pool.tile([P, D], f32)
                    for nd in range(ND):
                        ps = psB.tile([P, 512], f32, tag="psB")
                        for k in range(MF):
                            nc.tensor.matmul(
                                ps[:],
                                lhsT=hT[:, k, ts(mt, P)],
                                rhs=w2_sb[:, k, ts(nd, 512)],
                                start=(k == 0),
                                stop=(k == MF - 1),
                            )
                        src = ps[:]
                        if apply_b2:
                            tmp = out_pool.tile([P, 512], f32, tag="b2tmp")
                            nc.vector.tensor_tensor(
                                out=tmp[:],
                                in0=src,
                                in1=b2_sb[:, ts(nd, 512)],
                                op=mybir.AluOpType.add,
                            )
                            src = tmp[:]
                        nc.vector.tensor_scalar_mul(
                            out=o_sb[:, ts(nd, 512)],
                            in0=src,
                            scalar1=alpha_sb[:, gti:gti + 1],
                        )
                    nc.vector.tensor_tensor(
                        out=o_sb[:],
                        in0=o_sb[:],
                        in1=xd[:],
                        op=mybir.AluOpType.add,
                    )
                    nc.sync.dma_start(out=out_tiles[gti], in_=o_sb[:])
                c0 += Cc

    nc.compile()
    return nc


def _get_nc(C, apply_gb, apply_b1, apply_b2):
    key = (C, apply_gb, apply_b1, apply_b2)
    if key not in _NC_CACHE:
        _NC_CACHE[key] = _build_nc(C, apply_gb, apply_b1, apply_b2)
    return _NC_CACHE[key]


def kernel(input_features, centroids, ln_g, ln_b, w1, b1, w2, b2):
    global LAST_EXEC_TIME_NS, LAST_RESULTS
    from concourse.bass_utils import run_bass_kernel_spmd

    x = np.asarray(input_features, dtype=np.float32)
    cen = np.asarray(centroids, dtype=np.float32)
    ln_g = np.asarray(ln_g, dtype=np.float32)
    ln_b = np.asarray(ln_b, dtype=np.float32)
    w1 = np.asarray(w1, dtype=np.float32)
    b1 = np.asarray(b1, dtype=np.float32)
    w2 = np.asarray(w2, dtype=np.float32)
    b2 = np.asarray(b2, dtype=np.float32)

    xf = x.reshape(-1, D)
    n_tok = xf.shape[0]

    # host routing (float64: top-2 gaps are far above fp32 matmul noise)
    aff = xf.astype(np.float64) @ cen.T.astype(np.float64)
    eid = np.argmax(aff, axis=-1)
    dots = np.einsum(
        "td,td->t", xf.astype(np.float64), cen[eid].astype(np.float64)
    )
    alpha = (1.0 / (1.0 + np.exp(-dots))).astype(np.float32)

    idx = [np.nonzero(eid == e)[0] for e in range(E)]
    max_cnt = max(1, max(len(i) for i in idx))
    C = ((max_cnt + P - 1) // P) * P

    apply_gb = not (np.all(ln_g == 1.0) and np.all(ln_b == 0.0))
    apply_b1 = bool(np.any(b1 != 0.0))
    apply_b2 = bool(np.any(b2 != 0.0))

    nc = _get_nc(C, apply_gb, apply_b1, apply_b2)

    in_maps = []
    for e in range(E):
        pad = np.zeros(C, dtype=np.int64)
        pad[: len(idx[e])] = idx[e]
        im = {
            "x": np.ascontiguousarray(xf[pad]),
            "w1": w1[e].astype(ml_dtypes.bfloat16),
            "w2": w2[e].astype(ml_dtypes.bfloat16),
            "alpha_t": np.ascontiguousarray(alpha[pad].reshape(C // P, P).T),
        }
        if apply_b1:
            im["b1_t"] = np.ascontiguousarray(b1[e].reshape(F // P, P).T)
        if apply_gb:
            im["g_t"] = np.ascontiguousarray(ln_g[e].reshape(D // P, P).T)
            im["b_t"] = np.ascontiguousarray(ln_b[e].reshape(D // P, P).T)
        if apply_b2:
            im["b2"] = np.ascontiguousarray(b2[e].reshape(1, D))
        in_maps.append(im)

    want_trace = bool(int(os.environ.get("KERNEL_TRACE", "0")))
    if not want_trace:
        # The axon NTFF trace path needs antenv.axon_hooks, which this image
        # lacks unless test.py shims it; make sure an ambient BASS_TRACE env
        # can't crash the run.
        os.environ["BASS_NEVER_TRACE"] = "1"
    res = run_bass_kernel_spmd(
        nc,
        in_maps,
        list(range(E)),
        trace=want_trace,
    )
    LAST_EXEC_TIME_NS = res.exec_time_ns
    LAST_RESULTS = res

    out_full = np.empty((n_tok, D), dtype=np.float32)
    for e in range(E):
        if len(idx[e]):
            out_full[idx[e]] = res.results[e]["out"][: len(idx[e])]
    return out_full.reshape(x.shape)



# revision 2
# speedup vs baseline: 1.1904x; 1.1904x over previous
"""Expert-parallel MoE BaseLayer kernel for 8 Trainium2 NeuronCores.

Strategy (per the expert-parallel sharding hint):
  - Host: route tokens by argmax affinity (float64 numpy - the top-2 gaps are
    >>fp32 noise so this reproduces the reference's fp32 argmax), compute the
    sigmoid gate alpha on host, sort tokens by expert, pad each expert group
    to a common capacity C (multiple of 128).
  - Device (one Bass program, SPMD over 8 cores; core e holds expert e):
      LayerNorm (token-major) -> bf16 -> DRAM bounce -> XBAR-transposed load
      (D-major) -> ff1 (h^T = w1^T @ xln^T, PSUM-accumulated, bf16) ->
      fused relu+center+fp8-quantize -> ff2 in fp8 DoubleRow perf mode
      (2 K-tiles per pass = 157 TF/s) -> out = x + alpha * (ffn + bias).
  - fp8 error control: ff2 computes (h - c) @ (32*w2) with c ~= E[h] per
    column (analytic, c_f = ||w1[:,f]|| / sqrt(2*pi)). The exact c @ w2 + b2
    correction is computed on host in fp64 and folded into the bias; the
    1/32 descale is folded into alpha. Centering shrinks the fp8
    quantization range of both h and the h x dw2 error term.
  - Host: scatter per-expert outputs back to the original token order.
"""

import os

import numpy as np
import ml_dtypes

B, S, D, F, E = 8, 1024, 1024, 4096, 8
T = B * S
EPS = 1e-5
P = 128
CHUNK = 384  # tokens per pipeline chunk (<=512 for PSUM; 3 token-tiles)
S2 = 32.0    # fp8 scale on w2 (folded out via alpha and bias)

_NC_CACHE = {}
LAST_EXEC_TIME_NS = None
LAST_RESULTS = None


def _chunk_sizes(C):
    sizes = [CHUNK] * (C // CHUNK)
    if C % CHUNK:
        sizes.append(C % CHUNK)
    assert sum(sizes) == C and all(s % P == 0 for s in sizes)
    return sizes


def _build_nc(C, apply_gb, apply_b1):
    import concourse.bass as bass
    import concourse.tile as tile
    from concourse import bacc, mybir
    from concourse.bass import ts
    from concourse.masks import make_identity

    f32 = mybir.dt.float32
    bf16 = mybir.dt.bfloat16
    fp8 = mybir.dt.float8e4

    KD = D // P    # 8 k-tiles over D
    MF = F // P    # 32 f-tiles over F
    ND = D // 512  # 2 n-tiles over D for ff2
    n_tok_tiles = C // P
    chunks = _chunk_sizes(C)

    nc = bacc.Bacc()
    x_in = nc.declare_dram_parameter("x", [C, D], f32, isOutput=False)
    w1_in = nc.declare_dram_parameter("w1", [D, F], bf16, isOutput=False)
    w2_in = nc.declare_dram_parameter("w2", [F, D], fp8, isOutput=False)
    alpha_in = nc.declare_dram_parameter("alpha_t", [P, n_tok_tiles], f32, isOutput=False)
    c_in = nc.declare_dram_parameter("c_t", [P, MF], f32, isOutput=False)
    bias_in = nc.declare_dram_parameter("bias", [1, D], f32, isOutput=False)
    if apply_b1:
        b1_in = nc.declare_dram_parameter("b1_t", [P, MF], f32, isOutput=False)
    if apply_gb:
        g_in = nc.declare_dram_parameter("g_t", [P, KD], f32, isOutput=False)
        bb_in = nc.declare_dram_parameter("b_t", [P, KD], f32, isOutput=False)
    out_ext = nc.declare_dram_parameter("out", [C, D], f32, isOutput=True)

    x_tiles = x_in[:].rearrange("(t p) d -> t p d", p=P)
    out_tiles = out_ext[:].rearrange("(t p) d -> t p d", p=P)
    w1_view = w1_in[:].rearrange("(k p) f -> k p f", p=P)
    w2_view = w2_in[:].rearrange("(k p) d -> k p d", p=P)

    with tile.TileContext(nc) as tc:
        from contextlib import ExitStack

        with ExitStack() as ctx:
            singles = ctx.enter_context(tc.tile_pool(name="singles", bufs=1))
            xa_pool = ctx.enter_context(tc.tile_pool(name="xa", bufs=2))
            xn_pool = ctx.enter_context(tc.tile_pool(name="xn", bufs=2))
            st_pool = ctx.enter_context(tc.tile_pool(name="stats", bufs=4))
            xlnt_pool = ctx.enter_context(tc.tile_pool(name="xlnt", bufs=1))
            ht_pool = ctx.enter_context(tc.tile_pool(name="ht", bufs=1))
            xd_pool = ctx.enter_context(tc.tile_pool(name="xd", bufs=2))
            out_pool = ctx.enter_context(tc.tile_pool(name="outp", bufs=2))
            psA = ctx.enter_context(tc.tile_pool(name="psA", bufs=2, space="PSUM"))
            psT = ctx.enter_context(tc.tile_pool(name="psT", bufs=3, space="PSUM"))
            xf_pool = ctx.enter_context(tc.tile_pool(name="xf32", bufs=2))
            psB = ctx.enter_context(tc.tile_pool(name="psB", bufs=3, space="PSUM"))
            dram = ctx.enter_context(tc.tile_pool(name="dram", bufs=1, space="DRAM"))

            # --- small resident constants (cheap DMAs first) ------------
            alpha_sb = singles.tile([P, n_tok_tiles], f32)
            nc.sync.dma_start(out=alpha_sb[:], in_=alpha_in[:])
            c_sb = singles.tile([P, MF], f32)
            nc.sync.dma_start(out=c_sb[:], in_=c_in[:])
            eps_sb = singles.tile([P, 1], f32)
            nc.vector.memset(eps_sb, EPS)
            if apply_b1:
                b1_sb = singles.tile([P, MF], f32)
                nc.sync.dma_start(out=b1_sb[:], in_=b1_in[:])
            if apply_gb:
                g_sb = singles.tile([P, KD], f32)
                nc.sync.dma_start(out=g_sb[:], in_=g_in[:])
                b_sb = singles.tile([P, KD], f32)
                nc.sync.dma_start(out=b_sb[:], in_=bb_in[:])
            bias_sb = singles.tile([P, D], f32)
            nc.sync.dma_start(out=bias_sb[:], in_=bias_in[:].to_broadcast((P, D)))

            w1_sb = singles.tile([P, KD, F], bf16)
            w2_sb = singles.tile([P, MF, D], fp8)
            ident = singles.tile([P, P], f32)
            make_identity(nc, ident[:])

            # --- phase 0: LayerNorm + transpose for every chunk ---------
            # chunk 0's LN pipeline is emitted before the w1 bulk load so
            # its DMAs are not queued behind 8 MB of weights.
            xlnT = {}
            c0 = 0
            for ci, Cc in enumerate(chunks):
                pe_transpose = ci == 0
                if not pe_transpose:
                    xn_dram = dram.tile([Cc, D], bf16, tag=f"xnd{ci}")
                    xn_dview = xn_dram[:].rearrange("c (k p) -> c k p", p=P)
                xlnT_c = xlnt_pool.tile([P, KD, Cc], bf16, tag=f"xlnt{ci}")
                for tloc in range(Cc // P):
                    ti = c0 // P + tloc
                    x_sb = xa_pool.tile([P, D], f32)
                    nc.sync.dma_start(out=x_sb[:, :512], in_=x_tiles[ti][:, :512])
                    nc.sync.dma_start(out=x_sb[:, 512:], in_=x_tiles[ti][:, 512:])
                    stats = st_pool.tile([P, 2, 6], f32)
                    x_grp = x_sb[:].rearrange("p (s q) -> p s q", q=512)
                    for s in range(2):
                        nc.vector.bn_stats(out=stats[:, s, :], in_=x_grp[:, s, :])
                    mv = st_pool.tile([P, 2], f32)
                    nc.vector.bn_aggr(out=mv[:], in_=stats[:])
                    rstd = st_pool.tile([P, 1], f32)
                    nc.scalar.activation(
                        out=rstd[:],
                        in_=mv[:, 1:2],
                        func=mybir.ActivationFunctionType.Sqrt,
                        bias=eps_sb[:],
                        scale=1.0,
                    )
                    nc.vector.reciprocal(out=rstd[:], in_=rstd[:])
                    if pe_transpose:
                        # chunk 0: transpose on the (idle) PE instead of the
                        # DRAM bounce - keeps the ramp off the DMA queues.
                        xn32 = xf_pool.tile([P, D], f32)
                        nc.vector.tensor_scalar(
                            out=xn32[:],
                            in0=x_sb[:],
                            scalar1=mv[:, 0:1],
                            scalar2=rstd[:],
                            op0=mybir.AluOpType.subtract,
                            op1=mybir.AluOpType.mult,
                        )
                        for k in range(KD):
                            tps = psT.tile([P, P], f32, tag="psT")
                            nc.tensor.transpose(
                                tps[:], xn32[:, ts(k, P)], ident[:]
                            )
                            nc.vector.tensor_copy(
                                out=xlnT_c[:, k, tloc * P:(tloc + 1) * P],
                                in_=tps[:],
                            )
                    else:
                        xn_sb = xn_pool.tile([P, D], bf16)
                        nc.vector.tensor_scalar(
                            out=xn_sb[:],
                            in0=x_sb[:],
                            scalar1=mv[:, 0:1],
                            scalar2=rstd[:],
                            op0=mybir.AluOpType.subtract,
                            op1=mybir.AluOpType.mult,
                        )
                        nc.sync.dma_start(
                            out=xn_dram[tloc * P:(tloc + 1) * P, :], in_=xn_sb[:]
                        )
                if not pe_transpose:
                    # transposed load: [Cc, 128] -> [128, Cc] per D-tile
                    for k in range(KD):
                        nc.sync.dma_start(
                            out=xlnT_c[:, k, :], in_=xn_dview[:, k], transpose=True
                        )
                if apply_gb:
                    for k in range(KD):
                        nc.vector.tensor_scalar(
                            out=xlnT_c[:, k, :],
                            in0=xlnT_c[:, k, :],
                            scalar1=g_sb[:, k:k + 1],
                            scalar2=b_sb[:, k:k + 1],
                            op0=mybir.AluOpType.mult,
                            op1=mybir.AluOpType.add,
                        )
                xlnT[ci] = xlnT_c
                c0 += Cc
                if ci == 0:
                    # weight bulk loads after chunk 0's LN DMAs. w1 arrives in
                    # m-quarters (all k-rows of m 0..7 first, ...) so ff1's
                    # early m-sweeps start before the full 8 MB has landed.
                    FQ = F // 4
                    for q in range(4):
                        for k in range(KD):
                            nc.sync.dma_start(
                                out=w1_sb[:, k, q * FQ:(q + 1) * FQ],
                                in_=w1_view[k][:, q * FQ:(q + 1) * FQ],
                            )
                    for k in range(MF):
                        nc.sync.dma_start(out=w2_sb[:, k, :], in_=w2_view[k])

            # --- per chunk: ff1 -> relu+center->fp8 -> ff2 -> combine ---
            c0 = 0
            for ci, Cc in enumerate(chunks):
                n_mt = Cc // P
                # ff1: hc^T[f, t] = relu(z) - c for this chunk, in fp8
                hT = ht_pool.tile([P, MF, CHUNK], fp8, tag="ht")
                for m in range(MF):
                    ps = psA.tile([P, 512], f32, tag="psA")
                    for k in range(KD):
                        nc.tensor.matmul(
                            ps[:, :Cc],
                            lhsT=w1_sb[:, k, ts(m, P)],
                            rhs=xlnT[ci][:, k, :],
                            start=(k == 0),
                            stop=(k == KD - 1),
                        )
                    if apply_b1:
                        hf = xf_pool.tile([P, CHUNK], f32, tag="hb1")
                        nc.scalar.activation(
                            out=hf[:, :Cc],
                            in_=ps[:, :Cc],
                            func=mybir.ActivationFunctionType.Relu,
                            bias=b1_sb[:, m:m + 1],
                            scale=1.0,
                        )
                        nc.vector.tensor_scalar(
                            out=hT[:, m, :Cc],
                            in0=hf[:, :Cc],
                            scalar1=c_sb[:, m:m + 1],
                            scalar2=None,
                            op0=mybir.AluOpType.subtract,
                        )
                    else:
                        # fused: max(z, 0) - c, quantized to fp8
                        nc.vector.tensor_scalar(
                            out=hT[:, m, :Cc],
                            in0=ps[:, :Cc],
                            scalar1=0.0,
                            scalar2=c_sb[:, m:m + 1],
                            op0=mybir.AluOpType.max,
                            op1=mybir.AluOpType.subtract,
                        )

                # ff2 (fp8 DoubleRow) + combine, per 128-token tile:
                #   out = x + alpha * (ffn + bias)   [alpha,bias pre-scaled]
                for mt in range(n_mt):
                    gti = c0 // P + mt
                    xd = xd_pool.tile([P, D], f32)
                    nc.sync.dma_start(out=xd[:], in_=x_tiles[gti])
                    o_sb = out_pool.tile([P, D], f32)
                    for nd in range(ND):
                        ps = psB.tile([P, 512], f32, tag="psB")
                        for kk in range(MF // 2):
                            nc.tensor.matmul(
                                ps[:],
                                lhsT=hT[:, 2 * kk:2 * kk + 2, ts(mt, P)],
                                rhs=w2_sb[:, 2 * kk:2 * kk + 2, ts(nd, 512)],
                                start=(kk == 0),
                                stop=(kk == MF // 2 - 1),
                                perf_mode=mybir.MatmulPerfMode.DoubleRow,
                            )
                        tmp = out_pool.tile([P, 512], f32, tag="btmp")
                        nc.vector.tensor_tensor(
                            out=tmp[:],
                            in0=ps[:],
                            in1=bias_sb[:, ts(nd, 512)],
                            op=mybir.AluOpType.add,
                        )
                        nc.vector.tensor_scalar_mul(
                            out=o_sb[:, ts(nd, 512)],
                            in0=tmp[:],
                            scalar1=alpha_sb[:, gti:gti + 1],
                        )
                    nc.vector.tensor_tensor(
                        out=o_sb[:],
                        in0=o_sb[:],
                        in1=xd[:],
                        op=mybir.AluOpType.add,
                    )
                    nc.sync.dma_start(out=out_tiles[gti], in_=o_sb[:])
                c0 += Cc

    nc.compile()
    return nc


def _get_nc(C, apply_gb, apply_b1):
    key = (C, apply_gb, apply_b1)
    if key not in _NC_CACHE:
        _NC_CACHE[key] = _build_nc(C, apply_gb, apply_b1)
    return _NC_CACHE[key]


def kernel(input_features, centroids, ln_g, ln_b, w1, b1, w2, b2):
    global LAST_EXEC_TIME_NS, LAST_RESULTS
    from concourse.bass_utils import run_bass_kernel_spmd

    x = np.asarray(input_features, dtype=np.float32)
    cen = np.asarray(centroids, dtype=np.float32)
    ln_g = np.asarray(ln_g, dtype=np.float32)
    ln_b = np.asarray(ln_b, dtype=np.float32)
    w1 = np.asarray(w1, dtype=np.float32)
    b1 = np.asarray(b1, dtype=np.float32)
    w2 = np.asarray(w2, dtype=np.float32)
    b2 = np.asarray(b2, dtype=np.float32)

    xf = x.reshape(-1, D)
    n_tok = xf.shape[0]

    # host routing (float64: top-2 gaps are far above fp32 matmul noise)
    aff = xf.astype(np.float64) @ cen.T.astype(np.float64)
    eid = np.argmax(aff, axis=-1)
    dots = np.einsum(
        "td,td->t", xf.astype(np.float64), cen[eid].astype(np.float64)
    )
    alpha = (1.0 / (1.0 + np.exp(-dots))).astype(np.float32)

    idx = [np.nonzero(eid == e)[0] for e in range(E)]
    max_cnt = max(1, max(len(i) for i in idx))
    C = ((max_cnt + P - 1) // P) * P

    apply_gb = not (np.all(ln_g == 1.0) and np.all(ln_b == 0.0))
    apply_b1 = bool(np.any(b1 != 0.0))

    nc = _get_nc(C, apply_gb, apply_b1)

    in_maps = []
    for e in range(E):
        pad = np.zeros(C, dtype=np.int64)
        pad[: len(idx[e])] = idx[e]
        # analytic centering: c_f ~= E[relu(z_f)] = ||w1[:,f]|| / sqrt(2*pi)
        # (any c is algebraically exact; closeness to E[h] just shrinks the
        # fp8 quantization range). Exact c@w2 + b2 correction in fp64.
        c_e = (
            np.linalg.norm(w1[e].astype(np.float64), axis=0) / np.sqrt(2 * np.pi)
        )
        bias_e = (
            S2 * (c_e @ w2[e].astype(np.float64) + b2[e].astype(np.float64))
        ).astype(np.float32)
        im = {
            "x": np.ascontiguousarray(xf[pad]),
            "w1": w1[e].astype(ml_dtypes.bfloat16),
            "w2": (w2[e] * S2).astype(ml_dtypes.float8_e4m3),
            "alpha_t": np.ascontiguousarray(
                (alpha[pad] / S2).reshape(C // P, P).T
            ),
            "c_t": np.ascontiguousarray(
                c_e.astype(np.float32).reshape(F // P, P).T
            ),
            "bias": np.ascontiguousarray(bias_e.reshape(1, D)),
        }
        if apply_b1:
            im["b1_t"] = np.ascontiguousarray(b1[e].reshape(F // P, P).T)
        if apply_gb:
            im["g_t"] = np.ascontiguousarray(ln_g[e].reshape(D // P, P).T)
            im["b_t"] = np.ascontiguousarray(ln_b[e].reshape(D // P, P).T)
        in_maps.append(im)

    want_trace = bool(int(os.environ.get("KERNEL_TRACE", "0")))
    if not want_trace:
        # The axon NTFF trace path needs antenv.axon_hooks, which this image
        # lacks unless test.py shims it; make sure an ambient BASS_TRACE env
        # can't crash the run.
        os.environ["BASS_NEVER_TRACE"] = "1"
    res = run_bass_kernel_spmd(
        nc,
        in_maps,
        list(range(E)),
        trace=want_trace,
    )
    LAST_EXEC_TIME_NS = res.exec_time_ns
    LAST_RESULTS = res

    out_full = np.empty((n_tok, D), dtype=np.float32)
    for e in range(E):
        if len(idx[e]):
            out_full[idx[e]] = res.results[e]["out"][: len(idx[e])]
    return out_full.reshape(x.shape)


# revision 15
# speedup vs baseline: 1.2316x; 1.0345x over previous
"""Expert-parallel MoE BaseLayer kernel for 8 Trainium2 NeuronCores.

Strategy (per the expert-parallel sharding hint):
  - Host: route tokens by argmax affinity (float64 numpy - the top-2 gaps are
    >>fp32 noise so this reproduces the reference's fp32 argmax), compute the
    sigmoid gate alpha on host, sort tokens by expert, pad each expert group
    to a common capacity C (multiple of 128).
  - Device (one Bass program, SPMD over 8 cores; core e holds expert e):
      LayerNorm (token-major) -> bf16 -> DRAM bounce -> XBAR-transposed load
      (D-major) -> ff1 (h^T = w1^T @ xln^T, PSUM-accumulated, bf16) ->
      fused relu+center+fp8-quantize -> ff2 in fp8 DoubleRow perf mode
      (2 K-tiles per pass = 157 TF/s) -> out = x + alpha * (ffn + bias).
  - fp8 error control: ff2 computes (h - c) @ (32*w2) with c ~= E[h] per
    column (analytic, c_f = ||w1[:,f]|| / sqrt(2*pi)). The exact c @ w2 + b2
    correction is computed on host in fp64 and folded into the bias; the
    1/32 descale is folded into alpha. Centering shrinks the fp8
    quantization range of both h and the h x dw2 error term.
  - Scheduling: DMA emission is interleaved so token/bounce streams are not
    queued behind the 12 MB weight bulk (w1 in quarters just-in-time for
    ff1's m-sweep); chunk 2+ vector work is deferred past chunk 0's relu ops
    to avoid vector-queue head-of-line blocking; dummy PE matmuls at t~7us
    ramp the PE clock before the first real transpose.
  - Host: scatter per-expert outputs back to the original token order.
"""

import os

import numpy as np
import ml_dtypes

B, S, D, F, E = 8, 1024, 1024, 4096, 8
T = B * S
EPS = 1e-5
P = 128
CHUNK = 384  # tokens per pipeline chunk (<=512 for PSUM; 3 token-tiles)
S2 = 32.0    # fp8 scale on w2 (folded out via alpha and bias)
N_WARM = 11  # dummy PE matmuls to ramp the clock during the DMA/LN head

_NC_CACHE = {}
LAST_EXEC_TIME_NS = None
LAST_RESULTS = None


def _chunk_sizes(C):
    sizes = [CHUNK] * (C // CHUNK)
    if C % CHUNK:
        sizes.append(C % CHUNK)
    assert sum(sizes) == C and all(s % P == 0 for s in sizes)
    return sizes


def _build_nc(C, apply_gb, apply_b1):
    import concourse.bass as bass
    import concourse.tile as tile
    from concourse import bacc, mybir
    from concourse.bass import ts
    from concourse.masks import make_identity

    f32 = mybir.dt.float32
    bf16 = mybir.dt.bfloat16
    fp8 = mybir.dt.float8e4

    KD = D // P    # 8 k-tiles over D
    MF = F // P    # 32 f-tiles over F
    ND = D // 512  # 2 n-tiles over D for ff2
    n_tok_tiles = C // P
    chunks = _chunk_sizes(C)
    n_chunks = len(chunks)
    chunk_t0 = [sum(chunks[:i]) // P for i in range(n_chunks)]  # first tile idx

    nc = bacc.Bacc()
    x_in = nc.declare_dram_parameter("x", [C, D], f32, isOutput=False)
    w1_in = nc.declare_dram_parameter("w1", [D, F], bf16, isOutput=False)
    w2_in = nc.declare_dram_parameter("w2", [F, D], fp8, isOutput=False)
    alpha_in = nc.declare_dram_parameter("alpha_t", [P, n_tok_tiles], f32, isOutput=False)
    c_in = nc.declare_dram_parameter("c_t", [P, MF], f32, isOutput=False)
    bias_in = nc.declare_dram_parameter("bias", [1, D], f32, isOutput=False)
    if apply_b1:
        b1_in = nc.declare_dram_parameter("b1_t", [P, MF], f32, isOutput=False)
    if apply_gb:
        g_in = nc.declare_dram_parameter("g_t", [P, KD], f32, isOutput=False)
        bb_in = nc.declare_dram_parameter("b_t", [P, KD], f32, isOutput=False)
    out_ext = nc.declare_dram_parameter("out", [C, D], f32, isOutput=True)

    x_tiles = x_in[:].rearrange("(t p) d -> t p d", p=P)
    out_tiles = out_ext[:].rearrange("(t p) d -> t p d", p=P)
    w1_kview = w1_in[:].rearrange("(k p) f -> p k f", p=P)
    w2_kview = w2_in[:].rearrange("(k p) d -> p k d", p=P)

    with tile.TileContext(nc) as tc:
        from contextlib import ExitStack

        with ExitStack() as ctx:
            singles = ctx.enter_context(tc.tile_pool(name="singles", bufs=1))
            xa_pool = ctx.enter_context(tc.tile_pool(name="xa", bufs=5))
            xn_pool = ctx.enter_context(tc.tile_pool(name="xn", bufs=2))
            st_pool = ctx.enter_context(tc.tile_pool(name="stats", bufs=4))
            xlnt_pool = ctx.enter_context(tc.tile_pool(name="xlnt", bufs=1))
            ht_pool = ctx.enter_context(tc.tile_pool(name="ht", bufs=1))
            xd_pool = ctx.enter_context(tc.tile_pool(name="xd", bufs=4))
            out_pool = ctx.enter_context(tc.tile_pool(name="outp", bufs=2))
            psA = ctx.enter_context(tc.tile_pool(name="psA", bufs=3, space="PSUM"))
            psT = ctx.enter_context(tc.tile_pool(name="psT", bufs=2, space="PSUM"))
            xb_pool = ctx.enter_context(tc.tile_pool(name="xb", bufs=2))
            psB = ctx.enter_context(tc.tile_pool(name="psB", bufs=3, space="PSUM"))
            dram = ctx.enter_context(tc.tile_pool(name="dram", bufs=1, space="DRAM"))

            # --- constants built on-engine (no DMA) ---------------------
            eps_sb = singles.tile([P, 1], f32)
            nc.vector.memset(eps_sb, EPS)
            ident = singles.tile([P, P], f32)
            make_identity(nc, ident[:])
            ident_bf = singles.tile([P, P], bf16)
            make_identity(nc, ident_bf[:])
            warm_sb = singles.tile([P, 512], bf16)
            nc.gpsimd.memset(warm_sb, 0.0)

            # --- PE warmup: ramp the clock while DMAs/LN fill the head
            # (cycles the psB ring; WAW-serialized on the in-order PE) ------
            for i in range(N_WARM):
                warm_ps = psB.tile([P, 512], f32, tag="psB")
                nc.tensor.matmul(
                    warm_ps[:],
                    lhsT=ident_bf[:],
                    rhs=warm_sb[:],
                    start=True,
                    stop=True,
                )

            # declared-early tiles for deferred DMA emission
            alpha_sb = singles.tile([P, n_tok_tiles], f32)
            c_sb = singles.tile([P, MF], f32)
            bias_sb = singles.tile([P, D], f32)
            if apply_b1:
                b1_sb = singles.tile([P, MF], f32)
            if apply_gb:
                g_sb = singles.tile([P, KD], f32)
                b_sb = singles.tile([P, KD], f32)
            w1_sb = singles.tile([P, KD, F], bf16)
            w2_sb = singles.tile([P, MF, D], fp8)

            xlnT = {}
            xn_dram = {}
            x_pre = {}

            def emit_ln_chunk(ci, pe_transpose, x_dma_only=False):
                """LN pipeline for one chunk. pe_transpose: PE path (chunk 0);
                else writes bf16 bounce to DRAM. x_dma_only: only issue the
                x-tile loads (early DMA slot); the later full call picks the
                loaded tiles up from x_pre."""
                Cc = chunks[ci]
                t0 = chunk_t0[ci]
                if x_dma_only:
                    for tloc in range(Cc // P):
                        ti = t0 + tloc
                        x_sb = xa_pool.tile([P, D], f32)
                        nc.sync.dma_start(out=x_sb[:], in_=x_tiles[ti])
                        x_pre[ti] = x_sb
                    return
                if not pe_transpose:
                    xn_dram[ci] = dram.tile([Cc, D], bf16, tag=f"xnd{ci}", name=f"xnd{ci}")
                xlnT_c = xlnt_pool.tile([P, KD, Cc], bf16, tag=f"xlnt{ci}")
                for tloc in range(Cc // P):
                    ti = t0 + tloc
                    if ti in x_pre:
                        x_sb = x_pre.pop(ti)
                    else:
                        x_sb = xa_pool.tile([P, D], f32)
                        nc.sync.dma_start(out=x_sb[:], in_=x_tiles[ti])
                    stats = st_pool.tile([P, 2, 6], f32)
                    x_grp = x_sb[:].rearrange("p (s q) -> p s q", q=512)
                    for s in range(2):
                        nc.vector.bn_stats(out=stats[:, s, :], in_=x_grp[:, s, :])
                    mv = st_pool.tile([P, 2], f32)
                    nc.vector.bn_aggr(out=mv[:], in_=stats[:])
                    rstd = st_pool.tile([P, 1], f32)
                    nc.scalar.activation(
                        out=rstd[:],
                        in_=mv[:, 1:2],
                        func=mybir.ActivationFunctionType.Sqrt,
                        bias=eps_sb[:],
                        scale=1.0,
                    )
                    nc.vector.reciprocal(out=rstd[:], in_=rstd[:])
                    xnb = (xb_pool if pe_transpose else xn_pool).tile(
                        [P, D], f32 if pe_transpose else bf16, name="xnb"
                    )
                    nc.vector.tensor_scalar(
                        out=xnb[:],
                        in0=x_sb[:],
                        scalar1=mv[:, 0:1],
                        scalar2=rstd[:],
                        op0=mybir.AluOpType.subtract,
                        op1=mybir.AluOpType.mult,
                    )
                    if pe_transpose:
                        # chunk 0: transpose on the (ramping) PE instead of
                        # the DRAM bounce - keeps the head off the DMA queues.
                        for k in range(KD):
                            tps = psT.tile([P, P], f32, tag="psT")
                            nc.tensor.transpose(
                                tps[:], xnb[:, ts(k, P)], ident[:]
                            )
                            nc.scalar.copy(
                                out=xlnT_c[:, k, tloc * P:(tloc + 1) * P],
                                in_=tps[:],
                            )
                    else:
                        nc.sync.dma_start(
                            out=xn_dram[ci][tloc * P:(tloc + 1) * P, :], in_=xnb[:]
                        )
                if not pe_transpose:
                    # transposed load: [Cc, 128] -> [128, Cc] per D-tile
                    xn_dview = xn_dram[ci][:].rearrange("c (k p) -> c k p", p=P)
                    for k in range(KD):
                        nc.scalar.dma_start(
                            out=xlnT_c[:, k, :], in_=xn_dview[:, k], transpose=True
                        )
                if apply_gb:
                    for k in range(KD):
                        nc.vector.tensor_scalar(
                            out=xlnT_c[:, k, :],
                            in0=xlnT_c[:, k, :],
                            scalar1=g_sb[:, k:k + 1],
                            scalar2=b_sb[:, k:k + 1],
                            op0=mybir.AluOpType.mult,
                            op1=mybir.AluOpType.add,
                        )
                xlnT[ci] = xlnT_c

            def emit_w1_quarter(q):
                FQ = F // 4
                KH = KD // 2
                for kh in range(2):
                    nc.sync.dma_start(
                        out=w1_sb[:, kh * KH:(kh + 1) * KH, q * FQ:(q + 1) * FQ],
                        in_=w1_kview[:, kh * KH:(kh + 1) * KH, q * FQ:(q + 1) * FQ],
                    )

            # --- phase 0: interleave LN/bounce streams with weight bulk -
            # so neither the PE's weight feed nor the later chunks'
            # transposed loads are starved (DMA queues drain in order).
            # Chunk 1/2 vector work is NOT emitted here: it would sit ahead
            # of chunk 0's relu ops in the vector queue and head-of-line
            # block them (stalling ff1's PSUM recycling). Only their x
            # loads are issued; the LN ops are emitted mid-compute.
            emit_ln_chunk(0, pe_transpose=True)
            nc.sync.dma_start(out=c_sb[:], in_=c_in[:])
            if apply_b1:
                nc.sync.dma_start(out=b1_sb[:], in_=b1_in[:])
            if apply_gb:
                nc.sync.dma_start(out=g_sb[:], in_=g_in[:])
                nc.sync.dma_start(out=b_sb[:], in_=bb_in[:])
            emit_w1_quarter(0)
            emit_w1_quarter(1)
            if n_chunks > 1:
                emit_ln_chunk(1, pe_transpose=False, x_dma_only=True)
            emit_w1_quarter(2)
            emit_w1_quarter(3)
            if n_chunks > 2:
                emit_ln_chunk(2, pe_transpose=False, x_dma_only=True)
            for j in range(8):
                KJ = MF // 8
                nc.sync.dma_start(
                    out=w2_sb[:, j * KJ:(j + 1) * KJ, :],
                    in_=w2_kview[:, j * KJ:(j + 1) * KJ, :],
                )
            nc.gpsimd.dma_start(out=alpha_sb[:], in_=alpha_in[:])
            nc.gpsimd.dma_start(out=bias_sb[:], in_=bias_in[:].to_broadcast((P, D)))

            # preload chunk 0's combine-time x tiles (slack DMA window)
            xd_tiles = {}
            for mt in range(chunks[0] // P):
                xd = xd_pool.tile([P, D], f32)
                nc.gpsimd.dma_start(out=xd[:], in_=x_tiles[mt])
                xd_tiles[mt] = xd

            # --- per chunk: ff1 -> relu+center->fp8 -> ff2 -> combine ---
            for ci, Cc in enumerate(chunks):
                n_mt = Cc // P
                t0 = chunk_t0[ci]
                # ff1: hc^T[f, t] = relu(z) - c for this chunk, in fp8
                hT = ht_pool.tile([P, MF, CHUNK], fp8, tag="ht")
                for m in range(MF):
                    if m == 16 and ci == 0 and n_chunks > 1:
                        # chunk 1's LN pipeline: its vector-queue slot is
                        # behind chunk 0's first 16 relus (no head-of-line
                        # block) yet its transposed loads still complete
                        # well before ff1(chunk 1) needs them.
                        emit_ln_chunk(1, pe_transpose=False)
                    ps = psA.tile([P, 512], f32, tag="psA")
                    for k in range(KD):
                        nc.tensor.matmul(
                            ps[:, :Cc],
                            lhsT=w1_sb[:, k, ts(m, P)],
                            rhs=xlnT[ci][:, k, :],
                            start=(k == 0),
                            stop=(k == KD - 1),
                        )
                    if apply_b1:
                        hf = xb_pool.tile([P, CHUNK], f32, tag="hb1")
                        nc.scalar.activation(
                            out=hf[:, :Cc],
                            in_=ps[:, :Cc],
                            func=mybir.ActivationFunctionType.Relu,
                            bias=b1_sb[:, m:m + 1],
                            scale=1.0,
                        )
                        nc.vector.tensor_scalar(
                            out=hT[:, m, :Cc],
                            in0=hf[:, :Cc],
                            scalar1=c_sb[:, m:m + 1],
                            scalar2=None,
                            op0=mybir.AluOpType.subtract,
                        )
                    else:
                        # fused: max(z, 0) - c, quantized to fp8
                        nc.vector.tensor_scalar(
                            out=hT[:, m, :Cc],
                            in0=ps[:, :Cc],
                            scalar1=0.0,
                            scalar2=c_sb[:, m:m + 1],
                            op0=mybir.AluOpType.max,
                            op1=mybir.AluOpType.subtract,
                        )

                # ff2 (fp8 DoubleRow) + combine, per 128-token tile:
                #   out = x + alpha * (ffn + bias)   [alpha,bias pre-scaled]
                for mt in range(n_mt):
                    gti = t0 + mt
                    if gti in xd_tiles:
                        xd = xd_tiles.pop(gti)
                    else:
                        xd = xd_pool.tile([P, D], f32)
                        nc.gpsimd.dma_start(out=xd[:], in_=x_tiles[gti])
                    o_sb = out_pool.tile([P, D], f32)
                    for nd in range(ND):
                        ps = psB.tile([P, 512], f32, tag="psB")
                        for kk in range(MF // 2):
                            nc.tensor.matmul(
                                ps[:],
                                lhsT=hT[:, 2 * kk:2 * kk + 2, ts(mt, P)],
                                rhs=w2_sb[:, 2 * kk:2 * kk + 2, ts(nd, 512)],
                                start=(kk == 0),
                                stop=(kk == MF // 2 - 1),
                                perf_mode=mybir.MatmulPerfMode.DoubleRow,
                            )
                        tmp = out_pool.tile([P, 512], f32, tag="btmp")
                        nc.vector.tensor_tensor(
                            out=tmp[:],
                            in0=ps[:],
                            in1=bias_sb[:, ts(nd, 512)],
                            op=mybir.AluOpType.add,
                        )
                        nc.vector.tensor_scalar_mul(
                            out=o_sb[:, ts(nd, 512)],
                            in0=tmp[:],
                            scalar1=alpha_sb[:, gti:gti + 1],
                        )
                        nc.vector.tensor_tensor(
                            out=o_sb[:, ts(nd, 512)],
                            in0=o_sb[:, ts(nd, 512)],
                            in1=xd[:, ts(nd, 512)],
                            op=mybir.AluOpType.add,
                        )
                    nc.gpsimd.dma_start(out=out_tiles[gti], in_=o_sb[:])

                # deferred LN pipeline for chunk ci+2 (vector queue slot is
                # behind this chunk's relus, so it cannot head-of-line block
                # them; its transposed loads are still far ahead of need)
                if ci + 2 < n_chunks:
                    emit_ln_chunk(ci + 2, pe_transpose=False)

    nc.compile()
    return nc


def _get_nc(C, apply_gb, apply_b1):
    key = (C, apply_gb, apply_b1)
    if key not in _NC_CACHE:
        _NC_CACHE[key] = _build_nc(C, apply_gb, apply_b1)
    return _NC_CACHE[key]


def kernel(input_features, centroids, ln_g, ln_b, w1, b1, w2, b2):
    global LAST_EXEC_TIME_NS, LAST_RESULTS
    from concourse.bass_utils import run_bass_kernel_spmd

    x = np.asarray(input_features, dtype=np.float32)
    cen = np.asarray(centroids, dtype=np.float32)
    ln_g = np.asarray(ln_g, dtype=np.float32)
    ln_b = np.asarray(ln_b, dtype=np.float32)
    w1 = np.asarray(w1, dtype=np.float32)
    b1 = np.asarray(b1, dtype=np.float32)
    w2 = np.asarray(w2, dtype=np.float32)
    b2 = np.asarray(b2, dtype=np.float32)

    xf = x.reshape(-1, D)
    n_tok = xf.shape[0]

    # host routing (float64: top-2 gaps are far above fp32 matmul noise)
    aff = xf.astype(np.float64) @ cen.T.astype(np.float64)
    eid = np.argmax(aff, axis=-1)
    dots = np.einsum(
        "td,td->t", xf.astype(np.float64), cen[eid].astype(np.float64)
    )
    alpha = (1.0 / (1.0 + np.exp(-dots))).astype(np.float32)

    idx = [np.nonzero(eid == e)[0] for e in range(E)]
    max_cnt = max(1, max(len(i) for i in idx))
    C = ((max_cnt + P - 1) // P) * P

    apply_gb = not (np.all(ln_g == 1.0) and np.all(ln_b == 0.0))
    apply_b1 = bool(np.any(b1 != 0.0))

    nc = _get_nc(C, apply_gb, apply_b1)

    in_maps = []
    for e in range(E):
        pad = np.zeros(C, dtype=np.int64)
        pad[: len(idx[e])] = idx[e]
        # analytic centering: c_f ~= E[relu(z_f)] = ||w1[:,f]|| / sqrt(2*pi)
        # (any c is algebraically exact; closeness to E[h] just shrinks the
        # fp8 quantization range). Exact c@w2 + b2 correction in fp64.
        c_e = (
            np.linalg.norm(w1[e].astype(np.float64), axis=0) / np.sqrt(2 * np.pi)
        )
        bias_e = (
            S2 * (c_e @ w2[e].astype(np.float64) + b2[e].astype(np.float64))
        ).astype(np.float32)
        im = {
            "x": np.ascontiguousarray(xf[pad]),
            "w1": w1[e].astype(ml_dtypes.bfloat16),
            "w2": (w2[e] * S2).astype(ml_dtypes.float8_e4m3),
            "alpha_t": np.ascontiguousarray(
                (alpha[pad] / S2).reshape(C // P, P).T
            ),
            "c_t": np.ascontiguousarray(
                c_e.astype(np.float32).reshape(F // P, P).T
            ),
            "bias": np.ascontiguousarray(bias_e.reshape(1, D)),
        }
        if apply_b1:
            im["b1_t"] = np.ascontiguousarray(b1[e].reshape(F // P, P).T)
        if apply_gb:
            im["g_t"] = np.ascontiguousarray(ln_g[e].reshape(D // P, P).T)
            im["b_t"] = np.ascontiguousarray(ln_b[e].reshape(D // P, P).T)
        in_maps.append(im)

    want_trace = bool(int(os.environ.get("KERNEL_TRACE", "0")))
    if not want_trace:
        # The axon NTFF trace path needs antenv.axon_hooks, which this image
        # lacks unless test.py shims it; make sure an ambient BASS_TRACE env
        # can't crash the run.
        os.environ["BASS_NEVER_TRACE"] = "1"
    res = run_bass_kernel_spmd(
        nc,
        in_maps,
        list(range(E)),
        trace=want_trace,
    )
    LAST_EXEC_TIME_NS = res.exec_time_ns
    LAST_RESULTS = res

    out_full = np.empty((n_tok, D), dtype=np.float32)
    for e in range(E):
        if len(idx[e]):
            out_full[idx[e]] = res.results[e]["out"][: len(idx[e])]
    return out_full.reshape(x.shape)


# revision 16
# speedup vs baseline: 1.2625x; 1.0251x over previous
"""Expert-parallel MoE BaseLayer kernel for 8 Trainium2 NeuronCores.

Strategy (per the expert-parallel sharding hint):
  - Host: route tokens by argmax affinity (float64 numpy - the top-2 gaps are
    >>fp32 noise so this reproduces the reference's fp32 argmax), compute the
    sigmoid gate alpha on host, sort tokens by expert, pad each expert group
    to a common capacity C (multiple of 128).
  - Device (one Bass program, SPMD over 8 cores; core e holds expert e):
      LayerNorm (token-major) -> bf16 -> DRAM bounce -> XBAR-transposed load
      (D-major) -> ff1 (h^T = w1^T @ xln^T, PSUM-accumulated, bf16) ->
      fused relu+center+fp8-quantize -> ff2 in fp8 DoubleRow perf mode
      (2 K-tiles per pass = 157 TF/s) -> out = x + alpha * (ffn + bias).
  - fp8 error control: ff2 computes (h - c) @ (32*w2) with c ~= E[h] per
    column (analytic, c_f = ||w1[:,f]|| / sqrt(2*pi)). The exact c @ w2 + b2
    correction is computed on host in fp64 and folded into the bias; the
    1/32 descale is folded into alpha. Centering shrinks the fp8
    quantization range of both h and the h x dw2 error term.
  - Scheduling: DMA emission is interleaved so token/bounce streams are not
    queued behind the 12 MB weight bulk (w1 in quarters just-in-time for
    ff1's m-sweep); chunk 2+ vector work is deferred past chunk 0's relu ops
    to avoid vector-queue head-of-line blocking; dummy PE matmuls at t~7us
    ramp the PE clock before the first real transpose.
  - Host: scatter per-expert outputs back to the original token order.
"""

import os

import numpy as np
import ml_dtypes

B, S, D, F, E = 8, 1024, 1024, 4096, 8
T = B * S
EPS = 1e-5
P = 128
CHUNK = 384  # tokens per pipeline chunk (<=512 for PSUM; 3 token-tiles)
S2 = 32.0    # fp8 scale on w2 (folded out via alpha and bias)
N_WARM = 8   # dummy PE matmuls to ramp the clock during the DMA/LN head

_NC_CACHE = {}
LAST_EXEC_TIME_NS = None
LAST_RESULTS = None


def _chunk_sizes(C):
    sizes = [CHUNK] * (C // CHUNK)
    if C % CHUNK:
        sizes.append(C % CHUNK)
    assert sum(sizes) == C and all(s % P == 0 for s in sizes)
    return sizes


def _build_nc(C, apply_gb, apply_b1):
    import concourse.bass as bass
    import concourse.tile as tile
    from concourse import bacc, mybir
    from concourse.bass import ts
    from concourse.masks import make_identity

    f32 = mybir.dt.float32
    bf16 = mybir.dt.bfloat16
    fp8 = mybir.dt.float8e4

    KD = D // P    # 8 k-tiles over D
    MF = F // P    # 32 f-tiles over F
    ND = D // 512  # 2 n-tiles over D for ff2
    n_tok_tiles = C // P
    chunks = _chunk_sizes(C)
    n_chunks = len(chunks)
    chunk_t0 = [sum(chunks[:i]) // P for i in range(n_chunks)]  # first tile idx

    nc = bacc.Bacc()
    x_in = nc.declare_dram_parameter("x", [C, D], f32, isOutput=False)
    w1_in = nc.declare_dram_parameter("w1", [D, F], bf16, isOutput=False)
    w2_in = nc.declare_dram_parameter("w2", [F, D], fp8, isOutput=False)
    alpha_in = nc.declare_dram_parameter("alpha_t", [P, n_tok_tiles], f32, isOutput=False)
    c_in = nc.declare_dram_parameter("c_t", [P, MF], f32, isOutput=False)
    bias_in = nc.declare_dram_parameter("bias", [1, D], f32, isOutput=False)
    if apply_b1:
        b1_in = nc.declare_dram_parameter("b1_t", [P, MF], f32, isOutput=False)
    if apply_gb:
        g_in = nc.declare_dram_parameter("g_t", [P, KD], f32, isOutput=False)
        bb_in = nc.declare_dram_parameter("b_t", [P, KD], f32, isOutput=False)
    out_ext = nc.declare_dram_parameter("out", [C, D], f32, isOutput=True)

    x_tiles = x_in[:].rearrange("(t p) d -> t p d", p=P)
    out_tiles = out_ext[:].rearrange("(t p) d -> t p d", p=P)
    w1_kview = w1_in[:].rearrange("(k p) f -> p k f", p=P)
    w2_kview = w2_in[:].rearrange("(k p) d -> p k d", p=P)

    with tile.TileContext(nc) as tc:
        from contextlib import ExitStack

        with ExitStack() as ctx:
            singles = ctx.enter_context(tc.tile_pool(name="singles", bufs=1))
            xa_pool = ctx.enter_context(tc.tile_pool(name="xa", bufs=5))
            xn_pool = ctx.enter_context(tc.tile_pool(name="xn", bufs=2))
            st_pool = ctx.enter_context(tc.tile_pool(name="stats", bufs=4))
            xlnt_pool = ctx.enter_context(tc.tile_pool(name="xlnt", bufs=1))
            ht_pool = ctx.enter_context(tc.tile_pool(name="ht", bufs=1))
            xd_pool = ctx.enter_context(tc.tile_pool(name="xd", bufs=4))
            out_pool = ctx.enter_context(tc.tile_pool(name="outp", bufs=2))
            psA = ctx.enter_context(tc.tile_pool(name="psA", bufs=2, space="PSUM"))
            psT = ctx.enter_context(tc.tile_pool(name="psT", bufs=2, space="PSUM"))
            xb_pool = ctx.enter_context(tc.tile_pool(name="xb", bufs=2))
            psB = ctx.enter_context(tc.tile_pool(name="psB", bufs=3, space="PSUM"))
            psW = ctx.enter_context(tc.tile_pool(name="psW", bufs=1, space="PSUM"))
            dram = ctx.enter_context(tc.tile_pool(name="dram", bufs=1, space="DRAM"))

            # --- constants built on-engine (no DMA) ---------------------
            eps_sb = singles.tile([P, 1], f32)
            nc.vector.memset(eps_sb, EPS)
            ident = singles.tile([P, P], f32)
            make_identity(nc, ident[:])
            ident_bf = singles.tile([P, P], bf16)
            make_identity(nc, ident_bf[:])
            warm_sb = singles.tile([P, 512], bf16)
            nc.gpsimd.memset(warm_sb, 0.0)

            # --- PE warmup: ramp the clock while DMAs/LN fill the head --
            warm_ps = psW.tile([P, 512], f32)
            for i in range(N_WARM):
                nc.tensor.matmul(
                    warm_ps[:],
                    lhsT=ident_bf[:],
                    rhs=warm_sb[:],
                    start=True,
                    stop=True,
                )

            # declared-early tiles for deferred DMA emission
            alpha_sb = singles.tile([P, n_tok_tiles], f32)
            c_sb = singles.tile([P, MF], f32)
            bias_sb = singles.tile([P, D], f32)
            if apply_b1:
                b1_sb = singles.tile([P, MF], f32)
            if apply_gb:
                g_sb = singles.tile([P, KD], f32)
                b_sb = singles.tile([P, KD], f32)
            w1_sb = singles.tile([P, KD, F], bf16)
            w2_sb = singles.tile([P, MF, D], fp8)

            xlnT = {}
            xn_dram = {}
            x_pre = {}

            def emit_ln_chunk(ci, pe_transpose, x_dma_only=False):
                """LN pipeline for one chunk. pe_transpose: PE path (chunk 0);
                else writes bf16 bounce to DRAM. x_dma_only: only issue the
                x-tile loads (early DMA slot); the later full call picks the
                loaded tiles up from x_pre."""
                Cc = chunks[ci]
                t0 = chunk_t0[ci]
                if x_dma_only:
                    for tloc in range(Cc // P):
                        ti = t0 + tloc
                        x_sb = xa_pool.tile([P, D], f32)
                        nc.sync.dma_start(out=x_sb[:], in_=x_tiles[ti])
                        x_pre[ti] = x_sb
                    return
                if not pe_transpose:
                    xn_dram[ci] = dram.tile([Cc, D], bf16, tag=f"xnd{ci}", name=f"xnd{ci}")
                xlnT_c = xlnt_pool.tile([P, KD, Cc], bf16, tag=f"xlnt{ci}")
                for tloc in range(Cc // P):
                    ti = t0 + tloc
                    if ti in x_pre:
                        x_sb = x_pre.pop(ti)
                    else:
                        x_sb = xa_pool.tile([P, D], f32)
                        nc.sync.dma_start(out=x_sb[:], in_=x_tiles[ti])
                    stats = st_pool.tile([P, 2, 6], f32)
                    x_grp = x_sb[:].rearrange("p (s q) -> p s q", q=512)
                    for s in range(2):
                        nc.vector.bn_stats(out=stats[:, s, :], in_=x_grp[:, s, :])
                    mv = st_pool.tile([P, 2], f32)
                    nc.vector.bn_aggr(out=mv[:], in_=stats[:])
                    rstd = st_pool.tile([P, 1], f32)
                    nc.scalar.activation(
                        out=rstd[:],
                        in_=mv[:, 1:2],
                        func=mybir.ActivationFunctionType.Sqrt,
                        bias=eps_sb[:],
                        scale=1.0,
                    )
                    nc.vector.reciprocal(out=rstd[:], in_=rstd[:])
                    xnb = (xb_pool if pe_transpose else xn_pool).tile(
                        [P, D], f32 if pe_transpose else bf16, name="xnb"
                    )
                    nc.vector.tensor_scalar(
                        out=xnb[:],
                        in0=x_sb[:],
                        scalar1=mv[:, 0:1],
                        scalar2=rstd[:],
                        op0=mybir.AluOpType.subtract,
                        op1=mybir.AluOpType.mult,
                    )
                    if pe_transpose:
                        # chunk 0: transpose on the (ramping) PE instead of
                        # the DRAM bounce - keeps the head off the DMA queues.
                        for k in range(KD):
                            tps = psT.tile([P, P], f32, tag="psT")
                            nc.tensor.transpose(
                                tps[:], xnb[:, ts(k, P)], ident[:]
                            )
                            nc.vector.tensor_copy(
                                out=xlnT_c[:, k, tloc * P:(tloc + 1) * P],
                                in_=tps[:],
                            )
                    else:
                        nc.sync.dma_start(
                            out=xn_dram[ci][tloc * P:(tloc + 1) * P, :], in_=xnb[:]
                        )
                if not pe_transpose:
                    # transposed load: [Cc, 128] -> [128, Cc] per D-tile
                    xn_dview = xn_dram[ci][:].rearrange("c (k p) -> c k p", p=P)
                    for k in range(KD):
                        nc.scalar.dma_start(
                            out=xlnT_c[:, k, :], in_=xn_dview[:, k], transpose=True
                        )
                if apply_gb:
                    for k in range(KD):
                        nc.vector.tensor_scalar(
                            out=xlnT_c[:, k, :],
                            in0=xlnT_c[:, k, :],
                            scalar1=g_sb[:, k:k + 1],
                            scalar2=b_sb[:, k:k + 1],
                            op0=mybir.AluOpType.mult,
                            op1=mybir.AluOpType.add,
                        )
                xlnT[ci] = xlnT_c

            def emit_w1_quarter(q):
                FQ = F // 4
                KH = KD // 2
                for kh in range(2):
                    nc.sync.dma_start(
                        out=w1_sb[:, kh * KH:(kh + 1) * KH, q * FQ:(q + 1) * FQ],
                        in_=w1_kview[:, kh * KH:(kh + 1) * KH, q * FQ:(q + 1) * FQ],
                    )

            # --- phase 0: interleave LN/bounce streams with weight bulk -
            # so neither the PE's weight feed nor the later chunks'
            # transposed loads are starved (DMA queues drain in order).
            # Chunk 1/2 vector work is NOT emitted here: it would sit ahead
            # of chunk 0's relu ops in the vector queue and head-of-line
            # block them (stalling ff1's PSUM recycling). Only their x
            # loads are issued; the LN ops are emitted mid-compute.
            emit_ln_chunk(0, pe_transpose=True)
            nc.sync.dma_start(out=c_sb[:], in_=c_in[:])
            if apply_b1:
                nc.sync.dma_start(out=b1_sb[:], in_=b1_in[:])
            if apply_gb:
                nc.sync.dma_start(out=g_sb[:], in_=g_in[:])
                nc.sync.dma_start(out=b_sb[:], in_=bb_in[:])
            emit_w1_quarter(0)
            emit_w1_quarter(1)
            if n_chunks > 1:
                emit_ln_chunk(1, pe_transpose=False, x_dma_only=True)
            emit_w1_quarter(2)
            emit_w1_quarter(3)
            if n_chunks > 2:
                emit_ln_chunk(2, pe_transpose=False, x_dma_only=True)
            for j in range(8):
                KJ = MF // 8
                nc.sync.dma_start(
                    out=w2_sb[:, j * KJ:(j + 1) * KJ, :],
                    in_=w2_kview[:, j * KJ:(j + 1) * KJ, :],
                )
            nc.gpsimd.dma_start(out=alpha_sb[:], in_=alpha_in[:])
            nc.gpsimd.dma_start(out=bias_sb[:], in_=bias_in[:].to_broadcast((P, D)))

            # preload chunk 0's combine-time x tiles (slack DMA window)
            xd_tiles = {}
            for mt in range(chunks[0] // P):
                xd = xd_pool.tile([P, D], f32)
                nc.gpsimd.dma_start(out=xd[:], in_=x_tiles[mt])
                xd_tiles[mt] = xd

            # --- per chunk: ff1 -> relu+center->fp8 -> ff2 -> combine ---
            for ci, Cc in enumerate(chunks):
                n_mt = Cc // P
                t0 = chunk_t0[ci]
                # ff1: hc^T[f, t] = relu(z) - c for this chunk, in fp8
                hT = ht_pool.tile([P, MF, CHUNK], fp8, tag="ht")
                for m in range(MF):
                    if m == 16 and ci == 0 and n_chunks > 1:
                        # chunk 1's LN pipeline: its vector-queue slot is
                        # behind chunk 0's first 16 relus (no head-of-line
                        # block) yet its transposed loads still complete
                        # well before ff1(chunk 1) needs them.
                        emit_ln_chunk(1, pe_transpose=False)
                    ps = psA.tile([P, 512], f32, tag="psA")
                    for k in range(KD):
                        nc.tensor.matmul(
                            ps[:, :Cc],
                            lhsT=w1_sb[:, k, ts(m, P)],
                            rhs=xlnT[ci][:, k, :],
                            start=(k == 0),
                            stop=(k == KD - 1),
                        )
                    if apply_b1:
                        hf = xb_pool.tile([P, CHUNK], f32, tag="hb1")
                        nc.scalar.activation(
                            out=hf[:, :Cc],
                            in_=ps[:, :Cc],
                            func=mybir.ActivationFunctionType.Relu,
                            bias=b1_sb[:, m:m + 1],
                            scale=1.0,
                        )
                        nc.vector.tensor_scalar(
                            out=hT[:, m, :Cc],
                            in0=hf[:, :Cc],
                            scalar1=c_sb[:, m:m + 1],
                            scalar2=None,
                            op0=mybir.AluOpType.subtract,
                        )
                    else:
                        # fused: max(z, 0) - c, quantized to fp8
                        nc.vector.tensor_scalar(
                            out=hT[:, m, :Cc],
                            in0=ps[:, :Cc],
                            scalar1=0.0,
                            scalar2=c_sb[:, m:m + 1],
                            op0=mybir.AluOpType.max,
                            op1=mybir.AluOpType.subtract,
                        )

                # ff2 (fp8 DoubleRow) + combine, per 128-token tile:
                #   out = x + alpha * (ffn + bias)   [alpha,bias pre-scaled]
                for mt in range(n_mt):
                    gti = t0 + mt
                    if gti in xd_tiles:
                        xd = xd_tiles.pop(gti)
                    else:
                        xd = xd_pool.tile([P, D], f32)
                        nc.gpsimd.dma_start(out=xd[:], in_=x_tiles[gti])
                    o_sb = out_pool.tile([P, D], f32)
                    for nd in range(ND):
                        ps = psB.tile([P, 512], f32, tag="psB")
                        for kk in range(MF // 2):
                            nc.tensor.matmul(
                                ps[:],
                                lhsT=hT[:, 2 * kk:2 * kk + 2, ts(mt, P)],
                                rhs=w2_sb[:, 2 * kk:2 * kk + 2, ts(nd, 512)],
                                start=(kk == 0),
                                stop=(kk == MF // 2 - 1),
                                perf_mode=mybir.MatmulPerfMode.DoubleRow,
                            )
                        tmp = out_pool.tile([P, 512], f32, tag="btmp")
                        nc.vector.tensor_tensor(
                            out=tmp[:],
                            in0=ps[:],
                            in1=bias_sb[:, ts(nd, 512)],
                            op=mybir.AluOpType.add,
                        )
                        nc.vector.tensor_scalar_mul(
                            out=o_sb[:, ts(nd, 512)],
                            in0=tmp[:],
                            scalar1=alpha_sb[:, gti:gti + 1],
                        )
                        nc.vector.tensor_tensor(
                            out=o_sb[:, ts(nd, 512)],
                            in0=o_sb[:, ts(nd, 512)],
                            in1=xd[:, ts(nd, 512)],
                            op=mybir.AluOpType.add,
                        )
                    nc.gpsimd.dma_start(out=out_tiles[gti], in_=o_sb[:])

                # deferred LN pipeline for chunk ci+2 (vector queue slot is
                # behind this chunk's relus, so it cannot head-of-line block
                # them; its transposed loads are still far ahead of need)
                if ci + 2 < n_chunks:
                    emit_ln_chunk(ci + 2, pe_transpose=False)

    nc.compile()
    return nc


def _get_nc(C, apply_gb, apply_b1):
    key = (C, apply_gb, apply_b1)
    if key not in _NC_CACHE:
        _NC_CACHE[key] = _build_nc(C, apply_gb, apply_b1)
    return _NC_CACHE[key]


def kernel(input_features, centroids, ln_g, ln_b, w1, b1, w2, b2):
    global LAST_EXEC_TIME_NS, LAST_RESULTS
    from concourse.bass_utils import run_bass_kernel_spmd

    x = np.asarray(input_features, dtype=np.float32)
    cen = np.asarray(centroids, dtype=np.float32)
    ln_g = np.asarray(ln_g, dtype=np.float32)
    ln_b = np.asarray(ln_b, dtype=np.float32)
    w1 = np.asarray(w1, dtype=np.float32)
    b1 = np.asarray(b1, dtype=np.float32)
    w2 = np.asarray(w2, dtype=np.float32)
    b2 = np.asarray(b2, dtype=np.float32)

    xf = x.reshape(-1, D)
    n_tok = xf.shape[0]

    # host routing (float64: top-2 gaps are far above fp32 matmul noise)
    aff = xf.astype(np.float64) @ cen.T.astype(np.float64)
    eid = np.argmax(aff, axis=-1)
    dots = np.einsum(
        "td,td->t", xf.astype(np.float64), cen[eid].astype(np.float64)
    )
    alpha = (1.0 / (1.0 + np.exp(-dots))).astype(np.float32)

    idx = [np.nonzero(eid == e)[0] for e in range(E)]
    max_cnt = max(1, max(len(i) for i in idx))
    C = ((max_cnt + P - 1) // P) * P

    apply_gb = not (np.all(ln_g == 1.0) and np.all(ln_b == 0.0))
    apply_b1 = bool(np.any(b1 != 0.0))

    nc = _get_nc(C, apply_gb, apply_b1)

    in_maps = []
    for e in range(E):
        pad = np.zeros(C, dtype=np.int64)
        pad[: len(idx[e])] = idx[e]
        # analytic centering: c_f ~= E[relu(z_f)] = ||w1[:,f]|| / sqrt(2*pi)
        # (any c is algebraically exact; closeness to E[h] just shrinks the
        # fp8 quantization range). Exact c@w2 + b2 correction in fp64.
        c_e = (
            np.linalg.norm(w1[e].astype(np.float64), axis=0) / np.sqrt(2 * np.pi)
        )
        bias_e = (
            S2 * (c_e @ w2[e].astype(np.float64) + b2[e].astype(np.float64))
        ).astype(np.float32)
        im = {
            "x": np.ascontiguousarray(xf[pad]),
            "w1": w1[e].astype(ml_dtypes.bfloat16),
            "w2": (w2[e] * S2).astype(ml_dtypes.float8_e4m3),
            "alpha_t": np.ascontiguousarray(
                (alpha[pad] / S2).reshape(C // P, P).T
            ),
            "c_t": np.ascontiguousarray(
                c_e.astype(np.float32).reshape(F // P, P).T
            ),
            "bias": np.ascontiguousarray(bias_e.reshape(1, D)),
        }
        if apply_b1:
            im["b1_t"] = np.ascontiguousarray(b1[e].reshape(F // P, P).T)
        if apply_gb:
            im["g_t"] = np.ascontiguousarray(ln_g[e].reshape(D // P, P).T)
            im["b_t"] = np.ascontiguousarray(ln_b[e].reshape(D // P, P).T)
        in_maps.append(im)

    want_trace = bool(int(os.environ.get("KERNEL_TRACE", "0")))
    if not want_trace:
        # The axon NTFF trace path needs antenv.axon_hooks, which this image
        # lacks unless test.py shims it; make sure an ambient BASS_TRACE env
        # can't crash the run.
        os.environ["BASS_NEVER_TRACE"] = "1"
    res = run_bass_kernel_spmd(
        nc,
        in_maps,
        list(range(E)),
        trace=want_trace,
    )
    LAST_EXEC_TIME_NS = res.exec_time_ns
    LAST_RESULTS = res

    out_full = np.empty((n_tok, D), dtype=np.float32)
    for e in range(E):
        if len(idx[e]):
            out_full[idx[e]] = res.results[e]["out"][: len(idx[e])]
    return out_full.reshape(x.shape)


# revision 21
# speedup vs baseline: 1.3169x; 1.0431x over previous
"""Expert-parallel MoE BaseLayer kernel for 8 Trainium2 NeuronCores.

Strategy (per the expert-parallel sharding hint):
  - Host: route tokens by argmax affinity (float64 numpy - the top-2 gaps are
    >>fp32 noise so this reproduces the reference's fp32 argmax), compute the
    sigmoid gate alpha on host, sort tokens by expert, pad each expert group
    to a common capacity C (multiple of 128).
  - Device (one Bass program, SPMD over 8 cores; core e holds expert e):
      LayerNorm (token-major) -> bf16 -> DRAM bounce -> XBAR-transposed load
      (D-major) -> ff1 (h^T = w1^T @ xln^T, PSUM-accumulated, bf16) ->
      fused relu+center+fp8-quantize -> ff2 in fp8 DoubleRow perf mode
      (2 K-tiles per pass = 157 TF/s) -> out = x + alpha * (ffn + bias).
  - fp8 error control: ff2 computes (h - c) @ (32*w2) with c ~= E[h] per
    column (analytic, c_f = ||w1[:,f]|| / sqrt(2*pi)). The exact c @ w2 + b2
    correction is computed on host in fp64 and folded into the bias; the
    1/32 descale is folded into alpha. Centering shrinks the fp8
    quantization range of both h and the h x dw2 error term.
  - Scheduling: DMA emission is interleaved so token/bounce streams are not
    queued behind the 12 MB weight bulk (w1 in quarters just-in-time for
    ff1's m-sweep); chunk 2+ vector work is deferred past chunk 0's relu ops
    to avoid vector-queue head-of-line blocking; dummy PE matmuls at t~7us
    ramp the PE clock before the first real transpose.
  - Host: scatter per-expert outputs back to the original token order.
"""

import os

import numpy as np
import ml_dtypes

B, S, D, F, E = 8, 1024, 1024, 4096, 8
T = B * S
EPS = 1e-5
P = 128
CHUNK = 384  # tokens per pipeline chunk (<=512 for PSUM; 3 token-tiles)
S2 = 32.0    # fp8 scale on w2 (folded out via alpha and bias)
N_WARM = 15  # dummy PE matmuls to ramp the clock during the DMA/LN head

_NC_CACHE = {}
LAST_EXEC_TIME_NS = None
LAST_RESULTS = None


def _chunk_sizes(C):
    sizes = [CHUNK] * (C // CHUNK)
    if C % CHUNK:
        sizes.append(C % CHUNK)
    assert sum(sizes) == C
    # every chunk except the last must stay a multiple of P
    assert all(s % P == 0 for s in sizes[:-1])
    return sizes


def _tiles_of(chunks, ci):
    """[(global_tile_idx, row0, rows)] for chunk ci (last tile may be <P)."""
    t0 = sum(-(-c // P) for c in chunks[:ci])
    r0 = sum(chunks[:ci])
    out = []
    rem = chunks[ci]
    tloc = 0
    while rem > 0:
        pt = min(P, rem)
        out.append((t0 + tloc, r0 + tloc * P, pt))
        tloc += 1
        rem -= pt
    return out


def _build_nc(C, apply_gb, apply_b1):
    import concourse.bass as bass
    import concourse.tile as tile
    from concourse import bacc, mybir
    from concourse.bass import ts
    from concourse.masks import make_identity

    f32 = mybir.dt.float32
    bf16 = mybir.dt.bfloat16
    fp8 = mybir.dt.float8e4

    KD = D // P    # 8 k-tiles over D
    MF = F // P    # 32 f-tiles over F
    ND = D // 512  # 2 n-tiles over D for ff2
    n_tok_tiles = -(-C // P)
    chunks = _chunk_sizes(C)
    n_chunks = len(chunks)

    nc = bacc.Bacc()
    x_in = nc.declare_dram_parameter("x", [C, D], f32, isOutput=False)
    w1_in = nc.declare_dram_parameter("w1", [D, F], bf16, isOutput=False)
    w2_in = nc.declare_dram_parameter("w2", [F, D], fp8, isOutput=False)
    alpha_in = nc.declare_dram_parameter("alpha_t", [P, n_tok_tiles], f32, isOutput=False)
    c_in = nc.declare_dram_parameter("c_t", [P, MF], f32, isOutput=False)
    bias_in = nc.declare_dram_parameter("bias", [1, D], f32, isOutput=False)
    if apply_b1:
        b1_in = nc.declare_dram_parameter("b1_t", [P, MF], f32, isOutput=False)
    if apply_gb:
        g_in = nc.declare_dram_parameter("g_t", [P, KD], f32, isOutput=False)
        bb_in = nc.declare_dram_parameter("b_t", [P, KD], f32, isOutput=False)
    out_ext = nc.declare_dram_parameter("out", [C, D], f32, isOutput=True)

    x_rows = x_in[:]
    out_rows = out_ext[:]
    w1_kview = w1_in[:].rearrange("(k p) f -> p k f", p=P)
    w2_kview = w2_in[:].rearrange("(k p) d -> p k d", p=P)

    with tile.TileContext(nc) as tc:
        from contextlib import ExitStack

        with ExitStack() as ctx:
            singles = ctx.enter_context(tc.tile_pool(name="singles", bufs=1))
            xa_pool = ctx.enter_context(tc.tile_pool(name="xa", bufs=5))
            xn_pool = ctx.enter_context(tc.tile_pool(name="xn", bufs=2))
            st_pool = ctx.enter_context(tc.tile_pool(name="stats", bufs=4))
            xlnt_pool = ctx.enter_context(tc.tile_pool(name="xlnt", bufs=1))
            ht_pool = ctx.enter_context(tc.tile_pool(name="ht", bufs=1))
            xd_pool = ctx.enter_context(tc.tile_pool(name="xd", bufs=4))
            out_pool = ctx.enter_context(tc.tile_pool(name="outp", bufs=2))
            psA = ctx.enter_context(tc.tile_pool(name="psA", bufs=2, space="PSUM"))
            psT = ctx.enter_context(tc.tile_pool(name="psT", bufs=2, space="PSUM"))
            xb_pool = ctx.enter_context(tc.tile_pool(name="xb", bufs=2))
            psB = ctx.enter_context(tc.tile_pool(name="psB", bufs=3, space="PSUM"))
            psW = ctx.enter_context(tc.tile_pool(name="psW", bufs=1, space="PSUM"))
            dram = ctx.enter_context(tc.tile_pool(name="dram", bufs=1, space="DRAM"))

            # --- constants built on-engine (no DMA) ---------------------
            eps_sb = singles.tile([P, 1], f32)
            nc.vector.memset(eps_sb, EPS)
            ident = singles.tile([P, P], f32)
            make_identity(nc, ident[:])
            ident_bf = singles.tile([P, P], bf16)
            make_identity(nc, ident_bf[:])
            warm_sb = singles.tile([P, 512], bf16)
            nc.gpsimd.memset(warm_sb, 0.0)

            # --- PE warmup: ramp the clock while DMAs/LN fill the head --
            warm_ps = psW.tile([P, 512], f32)
            for i in range(N_WARM):
                nc.tensor.matmul(
                    warm_ps[:],
                    lhsT=ident_bf[:],
                    rhs=warm_sb[:],
                    start=True,
                    stop=True,
                )

            # declared-early tiles for deferred DMA emission
            alpha_sb = singles.tile([P, n_tok_tiles], f32)
            c_sb = singles.tile([P, MF], f32)
            bias_sb = singles.tile([P, D], f32)
            if apply_b1:
                b1_sb = singles.tile([P, MF], f32)
            if apply_gb:
                g_sb = singles.tile([P, KD], f32)
                b_sb = singles.tile([P, KD], f32)
            w1_sb = singles.tile([P, KD, F], bf16)
            w2_sb = singles.tile([P, MF, D], fp8)

            xlnT = {}
            xn_dram = {}
            x_pre = {}

            def emit_ln_chunk(ci, pe_transpose, x_dma_only=False):
                """LN pipeline for one chunk. pe_transpose: PE path (chunk 0);
                else writes bf16 bounce to DRAM. x_dma_only: only issue the
                x-tile loads (early DMA slot); the later full call picks the
                loaded tiles up from x_pre."""
                Cc = chunks[ci]
                tiles = _tiles_of(chunks, ci)
                if x_dma_only:
                    for ti, r0, pt in tiles:
                        x_sb = xa_pool.tile([P, D], f32)
                        nc.sync.dma_start(out=x_sb[:pt], in_=x_rows[r0:r0 + pt, :])
                        x_pre[ti] = x_sb
                    return
                if not pe_transpose:
                    xn_dram[ci] = dram.tile([Cc, D], bf16, tag=f"xnd{ci}", name=f"xnd{ci}")
                xlnT_c = xlnt_pool.tile([P, KD, Cc], bf16, tag=f"xlnt{ci}")
                for tloc, (ti, r0, pt) in enumerate(tiles):
                    if ti in x_pre:
                        x_sb = x_pre.pop(ti)
                    else:
                        x_sb = xa_pool.tile([P, D], f32)
                        nc.sync.dma_start(out=x_sb[:pt], in_=x_rows[r0:r0 + pt, :])
                    stats = st_pool.tile([P, 2, 6], f32)
                    x_grp = x_sb[:].rearrange("p (s q) -> p s q", q=512)
                    for s in range(2):
                        nc.vector.bn_stats(
                            out=stats[:pt, s, :], in_=x_grp[:pt, s, :]
                        )
                    mv = st_pool.tile([P, 2], f32)
                    nc.vector.bn_aggr(out=mv[:pt], in_=stats[:pt])
                    rstd = st_pool.tile([P, 1], f32)
                    nc.scalar.activation(
                        out=rstd[:pt],
                        in_=mv[:pt, 1:2],
                        func=mybir.ActivationFunctionType.Sqrt,
                        bias=eps_sb[:pt],
                        scale=1.0,
                    )
                    nc.vector.reciprocal(out=rstd[:pt], in_=rstd[:pt])
                    xnb = (xb_pool if pe_transpose else xn_pool).tile(
                        [P, D], f32 if pe_transpose else bf16, name="xnb"
                    )
                    nc.vector.tensor_scalar(
                        out=xnb[:pt],
                        in0=x_sb[:pt],
                        scalar1=mv[:pt, 0:1],
                        scalar2=rstd[:pt],
                        op0=mybir.AluOpType.subtract,
                        op1=mybir.AluOpType.mult,
                    )
                    if pe_transpose:
                        # chunk 0: transpose on the (ramping) PE instead of
                        # the DRAM bounce - keeps the head off the DMA queues.
                        for k in range(KD):
                            tps = psT.tile([P, P], f32, tag="psT")
                            nc.tensor.transpose(
                                tps[:, :pt], xnb[:pt, ts(k, P)], ident[:pt, :pt]
                            )
                            nc.vector.tensor_copy(
                                out=xlnT_c[:, k, tloc * P:tloc * P + pt],
                                in_=tps[:, :pt],
                            )
                    else:
                        nc.sync.dma_start(
                            out=xn_dram[ci][tloc * P:tloc * P + pt, :],
                            in_=xnb[:pt],
                        )
                if not pe_transpose:
                    # transposed load: [Cc, 128] -> [128, Cc] per D-tile
                    xn_dview = xn_dram[ci][:].rearrange("c (k p) -> c k p", p=P)
                    for k in range(KD):
                        nc.scalar.dma_start(
                            out=xlnT_c[:, k, :], in_=xn_dview[:, k], transpose=True
                        )
                if apply_gb:
                    for k in range(KD):
                        nc.vector.tensor_scalar(
                            out=xlnT_c[:, k, :],
                            in0=xlnT_c[:, k, :],
                            scalar1=g_sb[:, k:k + 1],
                            scalar2=b_sb[:, k:k + 1],
                            op0=mybir.AluOpType.mult,
                            op1=mybir.AluOpType.add,
                        )
                xlnT[ci] = xlnT_c

            def emit_w1_quarter(q):
                FQ = F // 4
                KH = KD // 2
                for kh in range(2):
                    nc.sync.dma_start(
                        out=w1_sb[:, kh * KH:(kh + 1) * KH, q * FQ:(q + 1) * FQ],
                        in_=w1_kview[:, kh * KH:(kh + 1) * KH, q * FQ:(q + 1) * FQ],
                    )

            # --- phase 0: interleave LN/bounce streams with weight bulk -
            # so neither the PE's weight feed nor the later chunks'
            # transposed loads are starved (DMA queues drain in order).
            # Chunk 1/2 vector work is NOT emitted here: it would sit ahead
            # of chunk 0's relu ops in the vector queue and head-of-line
            # block them (stalling ff1's PSUM recycling). Only their x
            # loads are issued; the LN ops are emitted mid-compute.
            emit_ln_chunk(0, pe_transpose=True)
            nc.sync.dma_start(out=c_sb[:], in_=c_in[:])
            if apply_b1:
                nc.sync.dma_start(out=b1_sb[:], in_=b1_in[:])
            if apply_gb:
                nc.sync.dma_start(out=g_sb[:], in_=g_in[:])
                nc.sync.dma_start(out=b_sb[:], in_=bb_in[:])
            emit_w1_quarter(0)
            emit_w1_quarter(1)
            if n_chunks > 1:
                emit_ln_chunk(1, pe_transpose=False, x_dma_only=True)
            emit_w1_quarter(2)
            emit_w1_quarter(3)
            if n_chunks > 2:
                emit_ln_chunk(2, pe_transpose=False, x_dma_only=True)
            for j in range(8):
                KJ = MF // 8
                nc.sync.dma_start(
                    out=w2_sb[:, j * KJ:(j + 1) * KJ, :],
                    in_=w2_kview[:, j * KJ:(j + 1) * KJ, :],
                )
            nc.gpsimd.dma_start(out=alpha_sb[:], in_=alpha_in[:])
            nc.gpsimd.dma_start(out=bias_sb[:], in_=bias_in[:].to_broadcast((P, D)))

            # preload chunk 0's combine-time x tiles (slack DMA window)
            xd_tiles = {}
            for gti, r0, pt in _tiles_of(chunks, 0):
                xd = xd_pool.tile([P, D], f32)
                nc.gpsimd.dma_start(out=xd[:pt], in_=x_rows[r0:r0 + pt, :])
                xd_tiles[gti] = xd

            # --- per chunk: ff1 -> relu+center->fp8 -> ff2 -> combine ---
            for ci, Cc in enumerate(chunks):
                # ff1: hc^T[f, t] = relu(z) - c for this chunk, in fp8
                hT = ht_pool.tile([P, MF, CHUNK], fp8, tag="ht")
                for m in range(MF):
                    if m == 16 and ci == 0 and n_chunks > 1:
                        # chunk 1's LN pipeline: its vector-queue slot is
                        # behind chunk 0's first 16 relus (no head-of-line
                        # block) yet its transposed loads still complete
                        # well before ff1(chunk 1) needs them.
                        emit_ln_chunk(1, pe_transpose=False)
                    ps = psA.tile([P, 512], f32, tag="psA")
                    for k in range(KD):
                        nc.tensor.matmul(
                            ps[:, :Cc],
                            lhsT=w1_sb[:, k, ts(m, P)],
                            rhs=xlnT[ci][:, k, :],
                            start=(k == 0),
                            stop=(k == KD - 1),
                        )
                    if apply_b1:
                        hf = xb_pool.tile([P, CHUNK], f32, tag="hb1")
                        nc.scalar.activation(
                            out=hf[:, :Cc],
                            in_=ps[:, :Cc],
                            func=mybir.ActivationFunctionType.Relu,
                            bias=b1_sb[:, m:m + 1],
                            scale=1.0,
                        )
                        nc.vector.tensor_scalar(
                            out=hT[:, m, :Cc],
                            in0=hf[:, :Cc],
                            scalar1=c_sb[:, m:m + 1],
                            scalar2=None,
                            op0=mybir.AluOpType.subtract,
                        )
                    else:
                        # fused: max(z, 0) - c, quantized to fp8
                        nc.vector.tensor_scalar(
                            out=hT[:, m, :Cc],
                            in0=ps[:, :Cc],
                            scalar1=0.0,
                            scalar2=c_sb[:, m:m + 1],
                            op0=mybir.AluOpType.max,
                            op1=mybir.AluOpType.subtract,
                        )

                # ff2 (fp8 DoubleRow) + combine, per 128-token tile:
                #   out = x + alpha * (ffn + bias)   [alpha,bias pre-scaled]
                for mt, (gti, r0, pt) in enumerate(_tiles_of(chunks, ci)):
                    if gti in xd_tiles:
                        xd = xd_tiles.pop(gti)
                    else:
                        xd = xd_pool.tile([P, D], f32)
                        nc.gpsimd.dma_start(out=xd[:pt], in_=x_rows[r0:r0 + pt, :])
                    o_sb = out_pool.tile([P, D], f32)
                    for nd in range(ND):
                        ps = psB.tile([P, 512], f32, tag="psB")
                        for kk in range(MF // 2):
                            nc.tensor.matmul(
                                ps[:pt, :],
                                lhsT=hT[:, 2 * kk:2 * kk + 2, mt * P:mt * P + pt],
                                rhs=w2_sb[:, 2 * kk:2 * kk + 2, ts(nd, 512)],
                                start=(kk == 0),
                                stop=(kk == MF // 2 - 1),
                                perf_mode=mybir.MatmulPerfMode.DoubleRow,
                            )
                        tmp = out_pool.tile([P, 512], f32, tag="btmp")
                        nc.vector.tensor_tensor(
                            out=tmp[:pt, :],
                            in0=ps[:pt, :],
                            in1=bias_sb[:pt, ts(nd, 512)],
                            op=mybir.AluOpType.add,
                        )
                        nc.vector.tensor_scalar_mul(
                            out=o_sb[:pt, ts(nd, 512)],
                            in0=tmp[:pt, :],
                            scalar1=alpha_sb[:pt, gti:gti + 1],
                        )
                        nc.vector.tensor_tensor(
                            out=o_sb[:pt, ts(nd, 512)],
                            in0=o_sb[:pt, ts(nd, 512)],
                            in1=xd[:pt, ts(nd, 512)],
                            op=mybir.AluOpType.add,
                        )
                    nc.gpsimd.dma_start(
                        out=out_rows[r0:r0 + pt, :], in_=o_sb[:pt]
                    )

                # deferred LN pipeline for chunk ci+2 (vector queue slot is
                # behind this chunk's relus, so it cannot head-of-line block
                # them; its transposed loads are still far ahead of need)
                if ci + 2 < n_chunks:
                    emit_ln_chunk(ci + 2, pe_transpose=False)

    nc.compile()
    return nc


def _get_nc(C, apply_gb, apply_b1):
    key = (C, apply_gb, apply_b1)
    if key not in _NC_CACHE:
        _NC_CACHE[key] = _build_nc(C, apply_gb, apply_b1)
    return _NC_CACHE[key]


def _alpha_table(a):
    n_tt = -(-len(a) // P)
    ap = np.zeros(n_tt * P, dtype=np.float32)
    ap[: len(a)] = a
    return np.ascontiguousarray(ap.reshape(n_tt, P).T)


def kernel(input_features, centroids, ln_g, ln_b, w1, b1, w2, b2):
    global LAST_EXEC_TIME_NS, LAST_RESULTS
    from concourse.bass_utils import run_bass_kernel_spmd

    x = np.asarray(input_features, dtype=np.float32)
    cen = np.asarray(centroids, dtype=np.float32)
    ln_g = np.asarray(ln_g, dtype=np.float32)
    ln_b = np.asarray(ln_b, dtype=np.float32)
    w1 = np.asarray(w1, dtype=np.float32)
    b1 = np.asarray(b1, dtype=np.float32)
    w2 = np.asarray(w2, dtype=np.float32)
    b2 = np.asarray(b2, dtype=np.float32)

    xf = x.reshape(-1, D)
    n_tok = xf.shape[0]

    # host routing (float64: top-2 gaps are far above fp32 matmul noise)
    aff = xf.astype(np.float64) @ cen.T.astype(np.float64)
    eid = np.argmax(aff, axis=-1)
    dots = np.einsum(
        "td,td->t", xf.astype(np.float64), cen[eid].astype(np.float64)
    )
    alpha = (1.0 / (1.0 + np.exp(-dots))).astype(np.float32)

    idx = [np.nonzero(eid == e)[0] for e in range(E)]
    max_cnt = max(1, max(len(i) for i in idx))
    C = ((max_cnt + 15) // 16) * 16  # last token tile may be partial

    apply_gb = not (np.all(ln_g == 1.0) and np.all(ln_b == 0.0))
    apply_b1 = bool(np.any(b1 != 0.0))

    nc = _get_nc(C, apply_gb, apply_b1)

    in_maps = []
    for e in range(E):
        pad = np.zeros(C, dtype=np.int64)
        pad[: len(idx[e])] = idx[e]
        # analytic centering: c_f ~= E[relu(z_f)] = ||w1[:,f]|| / sqrt(2*pi)
        # (any c is algebraically exact; closeness to E[h] just shrinks the
        # fp8 quantization range). Exact c@w2 + b2 correction in fp64.
        c_e = (
            np.linalg.norm(w1[e].astype(np.float64), axis=0) / np.sqrt(2 * np.pi)
        )
        bias_e = (
            S2 * (c_e @ w2[e].astype(np.float64) + b2[e].astype(np.float64))
        ).astype(np.float32)
        im = {
            "x": np.ascontiguousarray(xf[pad]),
            "w1": w1[e].astype(ml_dtypes.bfloat16),
            "w2": (w2[e] * S2).astype(ml_dtypes.float8_e4m3),
            "alpha_t": _alpha_table(alpha[pad] / S2),
            "c_t": np.ascontiguousarray(
                c_e.astype(np.float32).reshape(F // P, P).T
            ),
            "bias": np.ascontiguousarray(bias_e.reshape(1, D)),
        }
        if apply_b1:
            im["b1_t"] = np.ascontiguousarray(b1[e].reshape(F // P, P).T)
        if apply_gb:
            im["g_t"] = np.ascontiguousarray(ln_g[e].reshape(D // P, P).T)
            im["b_t"] = np.ascontiguousarray(ln_b[e].reshape(D // P, P).T)
        in_maps.append(im)

    want_trace = bool(int(os.environ.get("KERNEL_TRACE", "0")))
    if not want_trace:
        # The axon NTFF trace path needs antenv.axon_hooks, which this image
        # lacks unless test.py shims it; make sure an ambient BASS_TRACE env
        # can't crash the run.
        os.environ["BASS_NEVER_TRACE"] = "1"
    res = run_bass_kernel_spmd(
        nc,
        in_maps,
        list(range(E)),
        trace=want_trace,
    )
    LAST_EXEC_TIME_NS = res.exec_time_ns
    LAST_RESULTS = res

    out_full = np.empty((n_tok, D), dtype=np.float32)
    for e in range(E):
        if len(idx[e]):
            out_full[idx[e]] = res.results[e]["out"][: len(idx[e])]
    return out_full.reshape(x.shape)
